# revision 47
# baseline (speedup 1.0000x reference)
"""Multi-head attention (B=8, N=1024, C=768, H=12) on 8 Trainium2 NeuronCores.

Sharding: data-parallel, one batch element per core. Each core computes the
full attention block for its batch: QKV projection, per-head softmax(QK^T/8)V,
and the output projection, entirely on-chip (SBUF/PSUM).

Design, derived from perfetto/NTFF traces of earlier revisions:
  - TensorE (~123us of matmul streaming at 1 col/cycle @2.4GHz) and ScalarE
    (exp over 12 x 1M scores, ~100us at 1 elem/cycle/lane @1.2GHz) are
    co-critical; DVE/DMA/Sync hide behind them.
  - Everything is bf16 except PSUM accumulation and normalization scalars:
    halves input DMA and SBUF at ~5.6e-3 rel err vs the 2e-2 gate.
  - Inputs arrive host-side partition-interleaved ([128, KT*cols], fully
    contiguous per-partition lines) so each weight family is ONE ~700ns
    dma_start at full HBM rate; all input DMAs share one HWDGE queue in
    need-order (per-queue completion is ~FIFO, concurrent queues split
    bandwidth), with w_qkv columns pair-major so pair 0 lands first and the
    exp pipeline starts ~12us in.
  - Scores are computed transposed (S^T = K Q^T) in (64,128) tile mode with
    the two heads of a pair on distinct PE row groups running concurrently;
    exp(S^T) (with the 1/8 scale folded into the activation) feeds P@V
    directly as the moving operand; V tiles carry an appended ones-column so
    the P@V matmul's 65th row is the softmax denominator for free.
  - PSUM budget (the binding constraint): 4 banks score ping-pong + 2 banks
    P@V accumulators + 2 banks for a QKV/output-projection filler chain.
    The Tile scheduler drops filler groups into exp-bound PE gaps; score
    blocks carry high_priority so the static schedule interleaves them ahead
    of pending fillers. 24 stexp slots let P@V lag behind exp.
  - Sweep structure: j-outer over token halves. Sweep j0 interleaves the V
    projection and K/Q(j0)/K(j1) half-groups as fillers; Q(t)-j1 halves and
    the j0 output projection fill sweep j1. The j1 output projection is the
    tail: tiles 4,5 hold full accumulation groups open in the freed score
    psum (only their pair-5 matmul follows the last softmax), tiles 6,7
    split pairs 0-4 (early) from pair 5 (late); the last normalization's
    DMA-latency chain is bridged by scratch keep-warm matmuls so the PE's
    HAM clock gate (cold = 1.2GHz after ~3.4us idle) stays open. Warmup
    matmuls during the DMA head and an exp-table preload serve the same
    purpose at the start.
  - Normalization: reciprocal on a [128,4] DMA-fold of the denominator row
    (~180ns vs ~3us FD-bound), partition-broadcast back via a DRAM bounce,
    odd head moved to partitions 64:128 by SBUF-SBUF DMA.
  - y is written bf16 (host upcasts) to halve the output-DMA tail, split
    across the sync and scalar queues.

  - x and the later weight families are split across the sync and scalar
    HWDGE queues (need-order FIFO per queue, both run concurrently).

Measured: 189.5-194us fast-state (best 189,524ns) vs 219.9us for the v1
baseline; rel err 5.59e-3. The device alternates between a fast and a
power-downclocked state (~+25us) under sustained benchmarking.
"""

import sys

import numpy as np

if "/opt/trn_rl_repo" not in sys.path:
    sys.path.insert(0, "/opt/trn_rl_repo")

import ml_dtypes

BF16 = ml_dtypes.bfloat16

B = 8
N = 1024
C = 768
H = 12
D = 64
SCALE = D ** -0.5
KT = C // 128           # 6 contraction tiles over channels
NT = N // 128           # 8 token tiles
PAIRS = H // 2          # 6 head pairs
NWARM = 8               # HAM-warmup matmuls during the DMA head
SPLIT_PV = False        # split-contraction row-paired P@V

_CACHE = {}


def build_program(fast=True, nwarm=NWARM):
    import concourse.bacc as bacc
    import concourse.mybir as mybir
    import concourse.tile as tile

    f32 = mybir.dt.float32
    bf16 = mybir.dt.bfloat16
    u16 = mybir.dt.uint16
    Exp = mybir.ActivationFunctionType.Exp
    fm = bf16

    nc = bacc.Bacc("TRN2", target_bir_lowering=False, debug=False)

    # all inputs arrive host-interleaved as [128, KT*cols] (partition p, col
    # k*cols+c holds source row k*128+p): each family is ONE dma_start with
    # fully contiguous per-partition lines, so issue cost is ~700ns and the
    # transfer runs at full HBM rate. x is split by token half so the first
    # score block only waits for ~1.1MB.
    xT0_d = nc.dram_tensor("xT0", [128, KT * 512], fm, kind="ExternalInput")
    xT1_d = nc.dram_tensor("xT1", [128, KT * 512], fm, kind="ExternalInput")
    # QK weight columns pair-major: [Q0|K0|Q1|K1|...|Q5|K5], V natural
    wqk0_d = nc.dram_tensor("wqk0", [128, KT * 256], fm, kind="ExternalInput")
    wqkA_d = nc.dram_tensor("wqkA", [128, KT * 512], fm, kind="ExternalInput")
    wqkB_d = nc.dram_tensor("wqkB", [128, KT * 768], fm, kind="ExternalInput")
    wv_d = nc.dram_tensor("wv", [128, KT * C], fm, kind="ExternalInput")
    wp_d = nc.dram_tensor("wp", [128, KT * C], fm, kind="ExternalInput")
    bias_d = nc.dram_tensor("bias_rep", [128, C], f32, kind="ExternalInput")
    y_d = nc.dram_tensor("y", [N, C], fm, kind="ExternalOutput")

    mm = nc.tensor.matmul

    with tile.TileContext(nc) as tc:
        with tc.tile_pool(name="pers", bufs=1) as pers, \
             tc.tile_pool(name="cyc", bufs=2) as pB, \
             tc.tile_pool(name="dramb", bufs=4, space="DRAM") as pDr, \
             tc.tile_pool(name="ps_s", bufs=2, space="PSUM") as psS, \
             tc.tile_pool(name="ps_pv", bufs=1, space="PSUM") as psPV, \
             tc.tile_pool(name="ps_f", bufs=2, space="PSUM") as psF:

            # ---- persistent SBUF tiles (one per DMA family) ----
            xt_j = [pers.tile([128, KT, 512], fm, name=f"xtj{j}", tag=f"xtj{j}")
                    for j in range(2)]
            wqk0_a = pers.tile([128, KT, 256], fm, name="wqk0", tag="wqk0")
            wqk0 = [wqk0_a[:, k, :] for k in range(KT)]
            # pairs 1-2 and 3-5, split so early pairs arrive sooner
            wqkA_a = pers.tile([128, KT, 512], fm, name="wqkA", tag="wqkA")
            wqkA = [wqkA_a[:, k, :] for k in range(KT)]
            wqkB_a = pers.tile([128, KT, 768], fm, name="wqkB", tag="wqkB")
            wqkB = [wqkB_a[:, k, :] for k in range(KT)]
            wv_a = pers.tile([128, KT, C], fm, name="wv", tag="wv")
            wv = [wv_a[:, k, :] for k in range(KT)]
            wp_a = pers.tile([128, KT, C], fm, name="wp", tag="wp")
            wp = [wp_a[:, k, :] for k in range(KT)]
            bias_t = pers.tile([128, C], f32, name="bias_t", tag="bias_t")
            # Q^T/K^T tiles [d, n]: tile m holds heads 2m (parts 0:64) and
            # 2m+1 (parts 64:128); m 0..5 = Q pairs, 6..11 = K pairs.
            qkt = [pers.tile([128, N], fm, name=f"qkt{m}", tag=f"qkt{m}")
                   for m in range(2 * PAIRS)]
            # V tiles [n-tile, pair, 130]: per pair block [V_h0 |1| V_h1 |1];
            # ones cols at 64 and 129 feed the denominator row of P@V.
            vbuf = [pers.tile([128, PAIRS, 130], fm, name=f"vbuf{i}",
                              tag=f"vbuf{i}")
                    for i in range(NT)]
            aot = [pers.tile([128, N], fm, name=f"aot{t}", tag=f"aot{t}")
                   for t in range(PAIRS)]
            scr = pers.tile([128, 1152], fm, name="scr", tag="scr")
            pre_src = pers.tile([128, 8], f32, name="pre_src", tag="pre_src")
            pre_dst = pers.tile([128, 8], f32, name="pre_dst", tag="pre_dst")

            # ---- t~0: warmup + DMA issue. wqk0 (the 384KB that gates the
            # first score block) gets the scalar HWDGE queue to itself; x is
            # split across both queues; wp/bias ride gpsimd SWDGE since they
            # are not needed until the output projection. The exp-table
            # preload is emitted after the scalar DMAs so the ~2.7us table
            # load does not delay the wqk0 issue.
            nc.vector.memset(scr[:].bitcast(u16), 0)
            nc.vector.memset(pre_src[:], 0.0)

            def keep_warm(n):
                # 1024-col matmuls on scratch data: hold the HAM clock gate
                # open across known PE-idle stretches
                for _ in range(n):
                    ps = psS.tile([128, 1024], f32, name="s_ps", tag="s")
                    mm(ps[:, 0:512], scr[:, 0:128], scr[:, 128:640],
                       start=True, stop=True)

            def keep_warm_fill(n):
                for _ in range(n):
                    ps = psF.tile([128, 512], f32, name="fill", tag="fill")
                    mm(ps[:], scr[:, 0:128], scr[:, 128:640],
                       start=True, stop=True)

            keep_warm_fill(nwarm)

            def r6(dram_ap):
                return dram_ap.rearrange("p (k n) -> p k n", k=KT)

            # the gating prefix (wqk0+x) runs alone and first on the sync
            # queue (per-queue completion is ~FIFO); the later families are
            # split in half across the sync and scalar queues so both run
            # concurrently at aggregate bandwidth while preserving need-order
            # on each queue. The exp-table preload goes ahead of the scalar
            # halves.
            nc.scalar.activation(pre_dst[:], pre_src[:], Exp, scale=1.0)
            nc.sync.dma_start(wqk0_a[:], r6(wqk0_d[:]))
            nc.sync.dma_start(xt_j[0][:, 0:3, :], r6(xT0_d[:, 0:3 * 512]))
            nc.sync.dma_start(xt_j[0][:, 3:6, :], r6(xT0_d[:, 3 * 512:]))
            nc.sync.dma_start(xt_j[1][:], r6(xT1_d[:]))
            nc.sync.dma_start(wv_a[:, 0:3, :], r6(wv_d[:, 0:3 * C]))
            nc.scalar.dma_start(wv_a[:, 3:6, :], r6(wv_d[:, 3 * C:]))
            nc.sync.dma_start(wqkA_a[:, 0:3, :], r6(wqkA_d[:, 0:3 * 512]))
            nc.scalar.dma_start(wqkA_a[:, 3:6, :], r6(wqkA_d[:, 3 * 512:]))
            nc.sync.dma_start(wqkB_a[:, 0:3, :], r6(wqkB_d[:, 0:3 * 768]))
            nc.scalar.dma_start(wqkB_a[:, 3:6, :], r6(wqkB_d[:, 3 * 768:]))
            nc.sync.dma_start(wp_a[:], r6(wp_d[:]))
            nc.sync.dma_start(bias_t[:], bias_d[:])

            for i in range(NT):
                ones_ap = vbuf[i].rearrange("p a (t c) -> p a t c", c=65)[:, :, :, 64]
                nc.vector.memset(ones_ap.bitcast(u16), 0x3F80)  # bf16 1.0

            # ---- QKV projection emitters (psF filler chain) ----
            def wqk_slice(k, t, which):
                # permuted column block for pair t: [Q(128)|K(128)]
                if t == 0:
                    return wqk0[k][:, 128 * which:128 * (which + 1)]
                if t <= 2:
                    base = 256 * (t - 1) + 128 * which
                    return wqkA[k][:, base:base + 128]
                base = 256 * (t - 3) + 128 * which
                return wqkB[k][:, base:base + 128]

            def emit_qk_half(t, which, jh):
                # one [128, 512] psum group: Q (which=0) or K (which=1),
                # token half jh -> qkt[t or 6+t][:, 512*jh:]
                mtile = qkt[t] if which == 0 else qkt[PAIRS + t]
                ps = psF.tile([128, 512], f32, name="fill", tag="fill")
                for k in range(KT):
                    mm(ps[:], wqk_slice(k, t, which), xt_j[jh][:, k, :],
                       start=(k == 0), stop=(k == KT - 1))
                nc.vector.tensor_copy(mtile[:, 512 * jh:512 * (jh + 1)], ps[:])

            def emit_v(i):
                for c0, w in ((0, 512), (512, 256)):
                    ps = psF.tile([128, 512], f32, name="fill", tag="fill")
                    ioff = 128 * (i % 4)
                    for k in range(KT):
                        mm(ps[:, 0:w], xt_j[i // 4][:, k, ioff:ioff + 128],
                           wv[k][:, c0:c0 + w],
                           start=(k == 0), stop=(k == KT - 1))
                    # scatter heads: even -> cols 0:64, odd -> cols 65:129
                    v_view = ps[:, 0:w].rearrange("p (a t c) -> p a t c",
                                                  t=2, c=64)
                    pa0 = c0 // 128
                    npair = w // 128
                    nc.vector.tensor_copy(
                        vbuf[i][:, pa0:pa0 + npair, 0:64], v_view[:, :, 0, :])
                    nc.vector.tensor_copy(
                        vbuf[i][:, pa0:pa0 + npair, 65:129], v_view[:, :, 1, :])

            # ---- output projection (psF filler chain), one (i, chunk) unit ----
            yts = {}

            def get_yt(i):
                # bf16 output tile: halves the tail output-DMA volume
                if i not in yts:
                    yts[i] = pB.tile([128, C], fm, name="yt", tag="yt",
                                     bufs=5)
                return yts[i]

            ytf = {}

            def get_ytf(i):
                # f32 staging for the tail's split accumulation
                if i not in ytf:
                    ytf[i] = pB.tile([128, C], f32, name="ytf", tag="ytf",
                                     bufs=4)
                return ytf[i]

            def emit_proj_unit(i, c0, w):
                yto = get_yt(i)
                pp = psF.tile([128, 512], f32, name="fill", tag="fill")
                for k in range(KT):
                    mm(pp[:, 0:w], aot[k][:, 128 * i:128 * (i + 1)],
                       wp[k][:, c0:c0 + w],
                       start=(k == 0), stop=(k == KT - 1))
                nc.vector.tensor_add(yto[:, c0:c0 + w], pp[:, 0:w],
                                     bias_t[:, c0:c0 + w])
                nc.sync.dma_start(
                    y_d[128 * i:128 * (i + 1), c0:c0 + w], yto[:, c0:c0 + w])

            # ---- attention ----
            def emit_scores_act(t, j):
                qt, kt = qkt[t], qkt[PAIRS + t]
                stexps = []
                ctx = tc.high_priority()
                ctx.__enter__()
                for i in range(NT):
                    s_ps = psS.tile([128, 1024], f32, name="s_ps", tag="s")
                    for h in range(2):
                        # S^T[m, n] = sum_d K^T[d, m] Q^T[d, n]; h0/h1 use
                        # distinct PE row groups and run concurrently.
                        mm(s_ps[:, 512 * h:512 * (h + 1)],
                           kt[64 * h:64 * (h + 1), 128 * i:128 * (i + 1)],
                           qt[64 * h:64 * (h + 1), 512 * j:512 * (j + 1)],
                           start=True, stop=True)
                    stexp = pB.tile([128, 2, 512], fm, name="stexp",
                                    tag="stexp", bufs=24)
                    nc.scalar.activation(
                        stexp[:, :, :],
                        s_ps[:].rearrange("p (h n) -> p h n", h=2),
                        Exp, scale=SCALE)
                    stexps.append(stexp)
                ctx.__exit__(None, None, None)
                return stexps

            def emit_pv(t, stexps):
                # split-contraction P@V: per key tile, tokens 0:64 and 64:128
                # run on distinct PE row groups; the two heads accumulate in
                # distinct PSUM banks, so pairs execute concurrently and the
                # tile mode stays (64,128) — same as the score matmuls.
                pv_ps = [psPV.tile([65, 512], f32, name=f"pv{h}", tag=f"pv{h}")
                         for h in range(2)]
                if SPLIT_PV:
                    for i in range(NT):
                        st = stexps[i]
                        first = (i == 0)
                        last = (i == NT - 1)
                        mm(pv_ps[0][:], vbuf[i][0:64, t, 0:65],
                           st[0:64, 0, :], start=first, stop=False)
                        mm(pv_ps[1][:], vbuf[i][64:128, t, 65:130],
                           st[64:128, 1, :], start=first, stop=False)
                        mm(pv_ps[0][:], vbuf[i][64:128, t, 0:65],
                           st[64:128, 0, :], start=False, stop=last)
                        mm(pv_ps[1][:], vbuf[i][0:64, t, 65:130],
                           st[0:64, 1, :], start=False, stop=last)
                else:
                    for i in range(NT):
                        for h in range(2):
                            mm(pv_ps[h][:],
                               vbuf[i][:, t, 65 * h:65 * (h + 1)],
                               stexps[i][:, h, :],
                               start=(i == 0), stop=(i == NT - 1))
                return pv_ps

            def emit_norm(t, j, pv_ps):
                # phase-ordered: copies free the P@V banks immediately; the
                # DMA-latency-bound broadcasts/multiplies run later.
                stages = []
                for h in range(2):
                    stage = pB.tile([65, 512], f32, name="stage", tag="stage",
                                    bufs=4)
                    nc.vector.tensor_copy(stage[:], pv_ps[h][:])
                    stages.append(stage)
                dens = []
                for h in range(2):
                    # [1, 512] DVE reciprocal is FD-bound (~3us); DMA the
                    # denominator row into [128, 4] first where it is ~180ns.
                    den_t = pB.tile([128, 4], f32, name="den_t", tag="den_t",
                                    bufs=4)
                    nc.sync.dma_start(den_t[:], stages[h][64:65, :])
                    dens.append(den_t)
                rbs = []
                for h in range(2):
                    nc.vector.reciprocal(dens[h][:], dens[h][:])
                    dr2 = pDr.tile([1, 512], f32, name="dr2", tag="dr2")
                    nc.sync.dma_start(
                        dr2[:].rearrange("p (a b) -> (p a) b", a=128),
                        dens[h][:])
                    # partition-broadcast of the reciprocal row: SBUF APs
                    # cannot partition-broadcast, so bounce through DRAM.
                    rb = pB.tile([64, 512], f32, name="rb", tag="rb", bufs=4)
                    nc.sync.dma_start(rb[:], dr2[:].to_broadcast((64, 512)))
                    rbs.append(rb)
                nc.vector.tensor_mul(aot[t][0:64, 512 * j:512 * (j + 1)],
                                     stages[0][0:64, :], rbs[0][:])
                tmp = pB.tile([64, 512], fm, name="tmp1", tag="tmp1")
                nc.vector.tensor_mul(tmp[:], stages[1][0:64, :], rbs[1][:])
                # DVE lanes cannot shift partitions; DMA moves the odd head
                # into partitions 64:128.
                nc.sync.dma_start(aot[t][64:128, 512 * j:512 * (j + 1)],
                                  tmp[:])

            # ---- sweep j=0 ----
            emit_qk_half(0, 1, 0)   # K0 j0
            emit_qk_half(0, 0, 0)   # Q0 j0
            emit_qk_half(0, 1, 1)   # K0 j1
            def qk_triple(t):
                emit_qk_half(t, 1, 0)   # K(t) j0
                emit_qk_half(t, 0, 0)   # Q(t) j0
                emit_qk_half(t, 1, 1)   # K(t) j1

            for t in range(PAIRS):
                stexps = emit_scores_act(t, 0)
                # V must be fully emitted before the first P@V (emission
                # order defines dependencies); scores/ACT blocks carry
                # high_priority instead, so the scheduler interleaves them
                # ahead of pending filler groups
                if t == 0:
                    emit_v(0); emit_v(1)
                    qk_triple(1)
                    emit_v(2); emit_v(3)
                    qk_triple(2)
                    emit_v(4); emit_v(5)
                    qk_triple(3)
                    emit_v(6); emit_v(7)
                elif t <= 2:
                    qk_triple(t + 3)
                pv_ps = emit_pv(t, stexps)
                emit_norm(t, 0, pv_ps)
            emit_qk_half(0, 0, 1)   # Q0 j1

            # ---- sweep j=1: Q(t+1)-j1 + j=0 output projection as filler ----
            proj_units = [(i, c0, w) for i in range(4)
                          for c0, w in ((0, 512), (512, 256))]
            per_step = [2, 1, 1, 2, 1, 1]
            u = 0
            keep_warm_fill(6)
            for t in range(PAIRS):
                stexps = emit_scores_act(t, 1)
                pv_ps = emit_pv(t, stexps)
                emit_norm(t, 1, pv_ps)
                if t < PAIRS - 1:
                    emit_qk_half(t + 1, 0, 1)   # Q(t+1) j1
                for _ in range(per_step[t]):
                    emit_proj_unit(*proj_units[u])
                    u += 1

            # ---- tail: j=1 output projection. Tiles 4,5 hold full
            # accumulation groups open in the freed score-psum slots (only
            # their pair-5 matmul runs after the last normalization); tiles
            # 6,7 use the filler chain with a split A(pairs 0-4)/B(pair 5)
            # accumulation. Keep-warm matmuls target the freed P@V banks and
            # pad the PE through the last normalization's DMA chain.
            for i in (4, 5):
                tile_s = psS.tile([128, 1024], f32, name="s_ps", tag="s")
                for c0, w in ((0, 512), (512, 256)):
                    for k in range(KT):
                        mm(tile_s[:, c0:c0 + w],
                           aot[k][:, 128 * i:128 * (i + 1)],
                           wp[k][:, c0:c0 + w],
                           start=(k == 0), stop=(k == KT - 1))
                yto = get_yt(i)
                nc.vector.tensor_add(yto[:], tile_s[:, 0:C], bias_t[:])
                nc.sync.dma_start(y_d[128 * i:128 * (i + 1), :], yto[:])
            for i in (6, 7):
                for c0, w in ((0, 512), (512, 256)):
                    yt = get_ytf(i)
                    pp = psF.tile([128, 512], f32, name="fill", tag="fill")
                    for k in range(KT - 1):
                        mm(pp[:, 0:w], aot[k][:, 128 * i:128 * (i + 1)],
                           wp[k][:, c0:c0 + w],
                           start=(k == 0), stop=(k == KT - 2))
                    nc.vector.tensor_add(yt[:, c0:c0 + w], pp[:, 0:w],
                                         bias_t[:, c0:c0 + w])
            keep_warm_fill(24)
            for i in (6, 7):
                ytf_i = ytf[i]
                yto = get_yt(i)
                pp = psF.tile([128, 512], f32, name="fill", tag="fill")
                mm(pp[:, 0:512], aot[KT - 1][:, 128 * i:128 * (i + 1)],
                   wp[KT - 1][:, 0:512], start=True, stop=True)
                pp2 = psF.tile([128, 512], f32, name="fill", tag="fill")
                mm(pp2[:, 0:256], aot[KT - 1][:, 128 * i:128 * (i + 1)],
                   wp[KT - 1][:, 512:768], start=True, stop=True)
                nc.vector.tensor_add(yto[:, 0:512], ytf_i[:, 0:512],
                                     pp[:, 0:512])
                nc.vector.tensor_add(yto[:, 512:768], ytf_i[:, 512:768],
                                     pp2[:, 0:256])
                eng = nc.sync if i % 2 == 0 else nc.scalar
                eng.dma_start(y_d[128 * i:128 * (i + 1), :], yto[:])


    nc.compile()
    return nc


QK_PERM = np.concatenate(
    [np.concatenate([np.arange(128 * t, 128 * t + 128),
                     np.arange(C + 128 * t, C + 128 * t + 128)])
     for t in range(PAIRS)]
    + [np.arange(2 * C, 3 * C)])


def ilv(a):
    """[KT*128, cols] -> [128, KT*cols] partition-interleaved, contiguous."""
    a = np.ascontiguousarray(a)
    return np.ascontiguousarray(
        a.reshape(KT, 128, -1).transpose(1, 0, 2).reshape(128, -1))


def make_in_maps(x, w_qkv, w_proj, b_proj):
    wqkvT = np.asarray(w_qkv, dtype=np.float32).T[:, QK_PERM].astype(BF16)
    wprojT = np.asarray(w_proj, dtype=np.float32).T.astype(BF16)
    bias_rep = np.ascontiguousarray(
        np.broadcast_to(np.asarray(b_proj, dtype=np.float32), (128, C)))
    x = np.asarray(x, dtype=np.float32)
    shared = {
        "wqk0": ilv(wqkvT[:, 0:256]),
        "wqkA": ilv(wqkvT[:, 256:768]),
        "wqkB": ilv(wqkvT[:, 768:2 * C]),
        "wv": ilv(wqkvT[:, 2 * C:3 * C]),
        "wp": ilv(wprojT),
        "bias_rep": bias_rep,
    }
    out = []
    for b in range(B):
        xT = x[b].T.astype(BF16)
        m = {"xT0": ilv(xT[:, 0:512]), "xT1": ilv(xT[:, 512:N])}
        m.update(shared)
        out.append(m)
    return out


def kernel(x, w_qkv, w_proj, b_proj):
    from concourse.bass_utils import run_bass_kernel_spmd

    if "nc" not in _CACHE:
        _CACHE["nc"] = build_program()
    nc = _CACHE["nc"]

    in_maps = make_in_maps(x, w_qkv, w_proj, b_proj)
    res = run_bass_kernel_spmd(nc, in_maps, core_ids=list(range(B)))
    out = np.stack([res.results[b]["y"] for b in range(B)], axis=0)
    return out.astype(np.float32)
